# revision 9
# baseline (speedup 1.0000x reference)
"""Multi-head self-attention (RoPE + softmax + out-proj) for Trainium2,
sharded over 8 NeuronCores: data-parallel over batch (4) x tensor-parallel
over heads (2 groups of 8). Each core computes q/k/v projections for its
head group, attention, and a partial output projection; the host sums the
two partials per batch and adds the bias.

Per-core layout highlights:
  - All matmul operands are bf16 (host pre-casts inputs), which streams at
    the full 1 cycle/row PE rate and halves SBUF/DMA traffic; PSUM
    accumulation stays fp32.
  - x^T stays resident in SBUF (no DRAM bounce); v is projected straight
    into a resident SBUF tile [128, mb, head, 65] whose 65th column is
    pre-set to 1 so each PV matmul also accumulates the softmax denominator
    in PSUM row 64.
  - q/k are produced transposed ([head_dim, n]); RoPE's rotate_half is done
    with 32-partition shifted bf16 copies, sign folded into a host-negated
    sin table.
  - Scores are computed transposed (S^T[m, n]) with K=64 row-group-packed
    matmul pairs (two heads concurrently in the PE array); exp runs on the
    scalar engine straight out of PSUM in 1024-wide instructions.
  - The softmax normalize (reciprocal -> PE broadcast -> multiply) is
    deferred one quarter so the PE never waits on the DVE chain; the
    attention wave of pair p is software-pipelined with the projections of
    pair p+1 and the output projection of finished quarters.
"""

import numpy as np

import concourse.bass as bass
import concourse.mybir as mybir
import concourse.tile as tile

B, N, DIM, H, DH = 4, 2048, 1024, 16, 64
SCALE = DH**-0.5
N_CORES = 8
HG = 8  # heads per core
INNER = HG * DH  # 512, inner dim slice per core
PAIRS = INNER // 128  # 4 head pairs (=128-partition inner chunks)
NB = 4  # n blocks of 512
MB = 16  # m blocks of 128
KD = DIM // 128  # 8 contraction chunks

F32 = mybir.dt.float32
F32R = mybir.dt.float32r
BF16 = mybir.dt.bfloat16
EXP = mybir.ActivationFunctionType.Exp

MAX_WAITS = 1


def _split_excess_waits(nc):
    """This walrus build rejects >1 semaphore wait per instruction; hoist
    excess waits onto nops inserted before the instruction on its engine."""
    import bass_rust

    for f in nc.m.functions:
        for bb in f.blocks:
            il = bb.instructions
            i = 0
            while i < len(il):
                inst = il[i]
                si = inst.sync_info
                if si is not None and si.on_wait and len(si.on_wait) > MAX_WAITS:
                    waits = list(si.on_wait)
                    si.on_wait = waits[:MAX_WAITS]
                    rest = waits[MAX_WAITS:]
                    eng = nc.engines[inst.engine]
                    insert_at = i
                    for j in range(0, len(rest), MAX_WAITS):
                        b = eng.nop(nofuse=True, hint="wait_split")
                        ni = b.ins
                        tail = nc.cur_bb.bb.instructions
                        assert tail[-1] is ni
                        tail.pop()
                        nsi = ni.sync_info
                        if nsi is None:
                            ni.sync_info = bass_rust.SyncInfo(
                                on_wait=rest[j : j + MAX_WAITS], on_update=[]
                            )
                        else:
                            nsi.on_wait = rest[j : j + MAX_WAITS]
                        il.insert(insert_at, ni)
                        insert_at += 1
                        i += 1
                i += 1


class _FixedTileContext(tile.TileContext):
    def __exit__(self, exc_type, exc_val, exc_tb):
        res = super().__exit__(exc_type, exc_val, exc_tb)
        if exc_type is None:
            _split_excess_waits(self.nc)
        return res


def build_kernel():
    nc = bass.Bass()
    xT = nc.dram_tensor("xT", [DIM, N], BF16, kind="ExternalInput")
    wq = nc.dram_tensor("wq", [DIM, INNER], BF16, kind="ExternalInput")
    wk = nc.dram_tensor("wk", [DIM, INNER], BF16, kind="ExternalInput")
    wv = nc.dram_tensor("wv", [DIM, INNER], BF16, kind="ExternalInput")
    wo = nc.dram_tensor("wo", [INNER, DIM], BF16, kind="ExternalInput")
    cosT = nc.dram_tensor("cosT", [128, N], BF16, kind="ExternalInput")
    sinT = nc.dram_tensor("sinT", [128, N], BF16, kind="ExternalInput")
    out = nc.dram_tensor("out", [N, DIM], BF16, kind="ExternalOutput")

    xTr = xT.rearrange("(c p) n -> p c n", p=128)

    with _FixedTileContext(nc) as tc:
        with (
            tc.tile_pool(name="const", bufs=1) as cpool,
            tc.tile_pool(name="qk", bufs=1) as qkpool,
            tc.tile_pool(name="ps", space=bass.MemorySpace.PSUM, bufs=1) as ps,
            tc.tile_pool(name="io", bufs=1) as iopool,
        ):
            # ---- constants / resident tensors ----
            cos_t = cpool.tile([128, N], BF16, tag="cos")
            sin_t = cpool.tile([128, N], BF16, tag="sin")
            nc.sync.dma_start(cos_t[:], cosT[:])
            nc.sync.dma_start(sin_t[:], sinT[:])
            ones_f = cpool.tile([128, 64], F32, tag="onesf")
            nc.vector.memset(ones_f[:], 1.0)
            onesr = cpool.tile([128, 64], F32R, tag="onesr")
            nc.vector.tensor_copy(onesr[:], ones_f[:])
            # x^T resident in SBUF (bf16): DMA'd in 512-col chunks below
            x_sb = cpool.tile([128, KD, N], BF16, tag="xsb")
            # v resident in SBUF, [m-part, m-block, head, 65]; the 65th
            # column stays 1.0 so PV also accumulates the softmax denom
            v_sb = cpool.tile([128, MB, HG, 65], BF16, tag="vsb")
            nc.vector.memset(v_sb[:], 1.0)

            # ---- per-pair q/k projection blocks (emitted interleaved with
            #      the previous pair's attention so the PE never idles) ----
            def proj_pair_blocks(p):
                csl = slice(p * 128, (p + 1) * 128)
                wt = {}

                def load_w():
                    for nm, wd in (("q", wq), ("k", wk)):
                        t = iopool.tile(
                            [128, KD, 128], BF16, tag=f"w{nm}", bufs=1,
                            name=f"w{nm}_{p}",
                        )
                        nc.gpsimd.dma_start(
                            t[:], wd.rearrange("(c p) i -> p c i", p=128)[:, :, csl]
                        )
                        wt[nm] = t

                qT_t = qkpool.tile([128, N], BF16, tag="qT", bufs=2)
                kT_t = qkpool.tile([128, N], BF16, tag="kT", bufs=2)

                def block(nb, nm, tgt):
                    def emit():
                        nsl = slice(nb * 512, (nb + 1) * 512)
                        pq = ps.tile([128, 2, 512], F32, tag="s", bufs=3, name="pq")
                        for dc in range(KD):
                            nc.tensor.matmul(
                                pq[:, 0, :], wt[nm][:, dc, :], x_sb[:, dc, nsl],
                                start=(dc == 0), stop=(dc == KD - 1),
                            )
                        # rotate_half via 32-partition shifted copies; sign
                        # folded into sin_t (host negates low half rows)
                        q0 = iopool.tile([128, 512], BF16, tag="q0", bufs=2, name="q0")
                        nc.vector.tensor_copy(q0[:], pq[:, 0, :])
                        rot = iopool.tile([128, 512], BF16, tag="rot", bufs=2, name="rot")
                        for g in range(4):
                            dst = slice(g * 32, (g + 1) * 32)
                            ssrc = slice((g ^ 1) * 32, ((g ^ 1) + 1) * 32)
                            nc.vector.tensor_copy(rot[dst, :], q0[ssrc, :])
                        tmp = iopool.tile([128, 512], BF16, tag="tmp", bufs=2, name="tmp")
                        nc.vector.tensor_mul(tmp[:], rot[:], sin_t[:, nsl])
                        nc.vector.tensor_mul(tgt[:, nsl], q0[:], cos_t[:, nsl])
                        nc.vector.tensor_add(tgt[:, nsl], tgt[:, nsl], tmp[:])

                    return emit

                blocks = []
                for nb in range(NB):
                    blocks.append(block(nb, "q", qT_t))
                    blocks.append(block(nb, "k", kT_t))
                return load_w, blocks, qT_t, kT_t

            load_w0, blocks0, qT0, kT0 = proj_pair_blocks(0)
            load_w0()

            # ---- first pass over x: v projection (all heads) + pair-0 q/k ----
            with tc.tile_pool(name="vproj", bufs=1) as vpj:
                wv_t = vpj.tile([128, KD, INNER], BF16, tag="wv")
                wvr = wv.rearrange("(c p) i -> p c i", p=128)
                for nb in range(NB):
                    nsl = slice(nb * 512, (nb + 1) * 512)
                    if nb == 0:
                        for dc in range(KD):
                            nc.gpsimd.dma_start(wv_t[:, dc, :], wvr[:, dc, :])
                    nc.sync.dma_start(x_sb[:, :, nsl], xTr[:, :, nsl])
                    for sub in range(4):
                        pv = ps.tile([128, 512], F32, tag="s", bufs=3, name="pv")
                        m0 = nb * 512 + sub * 128
                        for dc in range(KD):
                            nc.tensor.matmul(
                                pv[:],
                                x_sb[:, dc, m0 : m0 + 128],
                                wv_t[:, dc, :],
                                start=(dc == 0),
                                stop=(dc == KD - 1),
                            )
                        nc.vector.tensor_copy(
                            v_sb[:, nb * 4 + sub, :, 0:64],
                            pv.rearrange("p (h d) -> p h d", h=HG),
                        )
                    blocks0[2 * nb]()
                    blocks0[2 * nb + 1]()

            pair_qk = {0: (qT0, kT0)}

            # ---- attention (pair p) interleaved with projections (p+1) ----
            with tc.tile_pool(name="attn", bufs=1) as at:
                otn = [
                    at.tile([128, 4, 512], BF16, tag=f"otn{p}", name=f"otn{p}")
                    for p in range(PAIRS)
                ]
                wo_h = []

                def load_wo():
                    for dh, wtag in ((0, "qT"), (1, "kT")):
                        woh = qkpool.tile(
                            [128, PAIRS, 512], BF16, tag=wtag, bufs=2,
                            name=f"wo_h{dh}",
                        )
                        nc.gpsimd.dma_start(
                            woh[:],
                            wo.rearrange("(c p) d -> p c d", p=128)[
                                :, :, dh * 512 : (dh + 1) * 512
                            ],
                        )
                        wo_h.append(woh)

                opq = []
                nmq = []

                def outproj_block(nb, dh):
                    def emit():
                        q4, r4 = divmod(nb, 4)
                        nsl = slice(nb * 128, (nb + 1) * 128)
                        po = ps.tile([128, 2, 512], F32, tag="s", bufs=3, name="po")
                        for c in range(PAIRS):
                            nc.tensor.matmul(
                                po[:, 0, :],
                                otn[c][:, q4, r4 * 128 : (r4 + 1) * 128],
                                wo_h[dh][:, c, :],
                                start=(c == 0),
                                stop=(c == PAIRS - 1),
                            )
                        ost = iopool.tile([128, 512], BF16, tag="ost", bufs=2, name="ost")
                        nc.any.tensor_copy(ost[:], po[:, 0, :])
                        nc.sync.dma_start(
                            out[nsl, dh * 512 : (dh + 1) * 512], ost[:]
                        )

                    return emit

                def outproj_quarter(q4):
                    # queue this quarter's out-projection; drained a few
                    # blocks at a time inside the next quarter's loop
                    for r4 in range(4):
                        for dh in range(2):
                            opq.append(outproj_block(q4 * 4 + r4, dh))

                for p in range(PAIRS):
                    qT_t, kT_t = pair_qk.pop(p)
                    if p == PAIRS - 1:
                        load_wo()
                    if p + 1 < PAIRS:
                        load_wn, blocks_n, qTn, kTn = proj_pair_blocks(p + 1)
                        load_wn()
                        pair_qk[p + 1] = (qTn, kTn)
                    else:
                        blocks_n = []
                    blk_i = 0
                    for f in range(2):
                        for sub in range(2):
                            n0 = f * 1024 + sub * 512
                            ot_ab = [
                                ps.tile([128, 512], F32, tag="ot", bufs=2, name=f"ot{jj}")
                                for jj in range(2)
                            ]
                            for mb2 in range(MB // 2):
                                s_tiles = []
                                for j in range(2):
                                    psl = slice(64 * j, 64 * (j + 1))
                                    s_t = ps.tile([128, 2, 512], F32, tag="s", bufs=3, name=f"s{j}")
                                    for hm in range(2):
                                        mb = 2 * mb2 + hm
                                        msl = slice(mb * 128, (mb + 1) * 128)
                                        nc.tensor.matmul(
                                            s_t[:, hm, :],
                                            kT_t[psl, msl],
                                            qT_t[psl, n0 : n0 + 512],
                                            start=True,
                                            stop=True,
                                        )
                                    s_tiles.append(s_t)
                                pts = []
                                for j in range(2):
                                    pt = at.tile([128, 2, 512], BF16, tag="pt", bufs=5, name=f"pt{j}")
                                    nc.scalar.activation(
                                        pt[:], s_tiles[j][:], EXP, scale=SCALE
                                    )
                                    pts.append(pt)
                                for j in range(2):
                                    for hm in range(2):
                                        mb = 2 * mb2 + hm
                                        nc.tensor.matmul(
                                            ot_ab[j][0:65, :],
                                            v_sb[:, mb, 2 * p + j, :],
                                            pts[j][:, hm, :],
                                            start=(mb == 0),
                                            stop=(mb == MB - 1),
                                        )
                                # previous quarter's deferred normalize: its
                                # reciprocal is long done by now, so the bcast
                                # MMs slot in without stalling. Both head
                                # halves must drain before any outproj pop
                                # below reads otn (write-after-read hazard).
                                if mb2 in (1, 2) and nmq:
                                    nmq.pop(0)()
                                # spread next pair's projection work through
                                # the attention chain to keep the PE dense
                                if mb2 % 2 == 1:
                                    if blk_i < len(blocks_n):
                                        blocks_n[blk_i]()
                                    blk_i += 1
                                    # in the last pair, spread the previous
                                    # quarter's output projection here too
                                    if mb2 >= 3:
                                        for _ in range(3):
                                            if opq:
                                                opq.pop(0)()
                            # spill OT accumulators to SBUF (frees the
                            # psum banks for the next quarter immediately)
                            osb = at.tile([65, 2, 512], BF16, tag="ots", bufs=4)
                            nc.vector.tensor_copy(osb[:, 0, :], ot_ab[0][0:65, :])
                            nc.vector.tensor_copy(osb[:, 1, :], ot_ab[1][0:65, :])
                            # denominators -> recip; the bcast MMs + muls are
                            # DEFERRED one quarter so the PE never waits here
                            rin = at.tile([33, 512], F32, tag="rin", bufs=2)
                            nc.vector.tensor_copy(rin[0:1, :], osb[64:65, 0, :])
                            nc.vector.tensor_copy(rin[32:33, :], osb[64:65, 1, :])
                            rec = at.tile([33, 512], F32R, tag="rec", bufs=2)
                            with nc.allow_low_precision(
                                reason="f32r reciprocal for softmax denom"
                            ):
                                # one op covers rows 0..32; rows 1-31 junk
                                nc.vector.reciprocal(rec[:], rin[:])

                            def norm_emit(j, p=p, f=f, sub=sub, osb=osb, rec=rec):
                                row = 32 * j
                                bc = ps.tile(
                                    [128, 2, 512], F32, tag="s", bufs=3,
                                    name=f"bc{j}",
                                )
                                nc.tensor.matmul(
                                    bc[0:64, 0, :],
                                    onesr[row : row + 1, :],
                                    rec[row : row + 1, :],
                                    start=True,
                                    stop=True,
                                )
                                nc.vector.tensor_mul(
                                    otn[p][64 * j : 64 * (j + 1), f * 2 + sub, :],
                                    osb[0:64, j, :],
                                    bc[0:64, 0, :],
                                )

                            nmq.append(lambda ne=norm_emit: ne(0))
                            nmq.append(lambda ne=norm_emit: ne(1))
                            if p == PAIRS - 1:
                                outproj_quarter(f * 2 + sub)
                while nmq:
                    nmq.pop(0)()
                while opq:
                    opq.pop(0)()

    return nc


_CACHED = {}


def _get_kernel():
    if "nc" not in _CACHED:
        _CACHED["nc"] = build_kernel()
    return _CACHED["nc"]


def kernel(x, rotary_emb_x, Wq, Wkv, Wo, bo):
    import ml_dtypes

    from concourse.bass_utils import run_bass_kernel_spmd

    BF = ml_dtypes.bfloat16

    x = np.asarray(x, np.float32)
    rope = np.asarray(rotary_emb_x, np.float32)
    Wq = np.asarray(Wq, np.float32)
    Wkv = np.asarray(Wkv, np.float32)
    Wo = np.asarray(Wo, np.float32)
    bo = np.asarray(bo, np.float32)

    cosT = np.ascontiguousarray(np.cos(rope).T)  # [64, N]
    sinT = np.ascontiguousarray(np.sin(rope).T)
    cosT2 = np.concatenate([cosT, cosT], axis=0)
    sinT2 = np.concatenate([sinT, sinT], axis=0)
    # fold rotate_half's sign into sin: the low half of each 64-row head
    # block multiplies -q_hi
    sinT2 = sinT2.copy()
    sinT2[0:32] = -sinT2[0:32]
    sinT2[64:96] = -sinT2[64:96]
    cosT2 = cosT2.astype(BF)
    sinT2 = sinT2.astype(BF)

    Wk_full = Wkv[:, : H * DH]
    Wv_full = Wkv[:, H * DH :]

    xTs = [np.ascontiguousarray(x[b].T).astype(BF) for b in range(B)]
    in_maps = []
    for core in range(N_CORES):
        b, hg = divmod(core, 2)
        isl = slice(hg * INNER, (hg + 1) * INNER)
        in_maps.append(
            {
                "xT": xTs[b],
                "wq": np.ascontiguousarray(Wq[:, isl]).astype(BF),
                "wk": np.ascontiguousarray(Wk_full[:, isl]).astype(BF),
                "wv": np.ascontiguousarray(Wv_full[:, isl]).astype(BF),
                "wo": np.ascontiguousarray(Wo[isl, :]).astype(BF),
                "cosT": cosT2,
                "sinT": sinT2,
            }
        )

    nc = _get_kernel()
    _CACHED["in_maps"] = in_maps
    res = run_bass_kernel_spmd(nc, in_maps, list(range(N_CORES)))
    outs = [
        np.asarray(res.results[i]["out"]).astype(np.float32)
        for i in range(N_CORES)
    ]
    full = np.stack(
        [outs[2 * b] + outs[2 * b + 1] + bo for b in range(B)], axis=0
    )
    return full


# revision 10
# speedup vs baseline: 1.5899x; 1.5899x over previous
"""Multi-head self-attention (RoPE + softmax + out-proj) for Trainium2,
sharded over 8 NeuronCores: data-parallel over batch (4) x tensor-parallel
over heads (2 groups of 8). Each core computes q/k/v projections for its
head group, attention, and a partial output projection; the host sums the
two partials per batch and adds the bias.

Per-core layout highlights:
  - All matmul operands are bf16 (host pre-casts inputs), which streams at
    the full 1 cycle/row PE rate and halves SBUF/DMA traffic; PSUM
    accumulation stays fp32.
  - x^T stays resident in SBUF (no DRAM bounce); v is projected straight
    into a resident SBUF tile [128, mb, head, 65] whose 65th column is
    pre-set to 1 so each PV matmul also accumulates the softmax denominator
    in PSUM row 64.
  - q/k are produced transposed ([head_dim, n]); RoPE's rotate_half is done
    with 32-partition shifted bf16 copies, sign folded into a host-negated
    sin table.
  - Scores are computed transposed (S^T[m, n]) with K=64 row-group-packed
    matmul pairs (two heads concurrently in the PE array); exp runs on the
    scalar engine straight out of PSUM in 1024-wide instructions.
  - The softmax normalize (reciprocal -> PE broadcast -> multiply) is
    deferred one quarter so the PE never waits on the DVE chain; the
    attention wave of pair p is software-pipelined with the projections of
    pair p+1 and the output projection of finished quarters.
"""

import numpy as np

import concourse.bass as bass
import concourse.mybir as mybir
import concourse.tile as tile

B, N, DIM, H, DH = 4, 2048, 1024, 16, 64
SCALE = DH**-0.5
N_CORES = 8
HG = 8  # heads per core
INNER = HG * DH  # 512, inner dim slice per core
PAIRS = INNER // 128  # 4 head pairs (=128-partition inner chunks)
NB = 4  # n blocks of 512
MB = 16  # m blocks of 128
KD = DIM // 128  # 8 contraction chunks

F32 = mybir.dt.float32
F32R = mybir.dt.float32r
BF16 = mybir.dt.bfloat16
EXP = mybir.ActivationFunctionType.Exp

MAX_WAITS = 1


def _split_excess_waits(nc):
    """This walrus build rejects >1 semaphore wait per instruction; hoist
    excess waits onto nops inserted before the instruction on its engine."""
    import bass_rust

    for f in nc.m.functions:
        for bb in f.blocks:
            il = bb.instructions
            i = 0
            while i < len(il):
                inst = il[i]
                si = inst.sync_info
                if si is not None and si.on_wait and len(si.on_wait) > MAX_WAITS:
                    waits = list(si.on_wait)
                    si.on_wait = waits[:MAX_WAITS]
                    rest = waits[MAX_WAITS:]
                    eng = nc.engines[inst.engine]
                    insert_at = i
                    for j in range(0, len(rest), MAX_WAITS):
                        b = eng.nop(nofuse=True, hint="wait_split")
                        ni = b.ins
                        tail = nc.cur_bb.bb.instructions
                        assert tail[-1] is ni
                        tail.pop()
                        nsi = ni.sync_info
                        if nsi is None:
                            ni.sync_info = bass_rust.SyncInfo(
                                on_wait=rest[j : j + MAX_WAITS], on_update=[]
                            )
                        else:
                            nsi.on_wait = rest[j : j + MAX_WAITS]
                        il.insert(insert_at, ni)
                        insert_at += 1
                        i += 1
                i += 1


class _FixedTileContext(tile.TileContext):
    def __exit__(self, exc_type, exc_val, exc_tb):
        res = super().__exit__(exc_type, exc_val, exc_tb)
        if exc_type is None:
            _split_excess_waits(self.nc)
        return res


def build_kernel():
    nc = bass.Bass()
    xT = nc.dram_tensor("xT", [DIM, N], BF16, kind="ExternalInput")
    wq = nc.dram_tensor("wq", [DIM, INNER], BF16, kind="ExternalInput")
    wk = nc.dram_tensor("wk", [DIM, INNER], BF16, kind="ExternalInput")
    wv = nc.dram_tensor("wv", [DIM, INNER], BF16, kind="ExternalInput")
    wo = nc.dram_tensor("wo", [INNER, DIM], BF16, kind="ExternalInput")
    cosT = nc.dram_tensor("cosT", [128, N], BF16, kind="ExternalInput")
    sinT = nc.dram_tensor("sinT", [128, N], BF16, kind="ExternalInput")
    out = nc.dram_tensor("out", [N, DIM], BF16, kind="ExternalOutput")

    xTr = xT.rearrange("(c p) n -> p c n", p=128)

    with _FixedTileContext(nc) as tc:
        with (
            tc.tile_pool(name="const", bufs=1) as cpool,
            tc.tile_pool(name="qk", bufs=1) as qkpool,
            tc.tile_pool(name="ps", space=bass.MemorySpace.PSUM, bufs=1) as ps,
            tc.tile_pool(name="io", bufs=1) as iopool,
        ):
            # ---- constants / resident tensors ----
            cos_t = cpool.tile([128, N], BF16, tag="cos")
            sin_t = cpool.tile([128, N], BF16, tag="sin")
            nc.sync.dma_start(cos_t[:], cosT[:])
            nc.sync.dma_start(sin_t[:], sinT[:])
            ones_f = cpool.tile([128, 64], F32, tag="onesf")
            nc.vector.memset(ones_f[:], 1.0)
            onesr = cpool.tile([128, 64], F32R, tag="onesr")
            nc.vector.tensor_copy(onesr[:], ones_f[:])
            # x^T resident in SBUF (bf16): DMA'd in 512-col chunks below
            x_sb = cpool.tile([128, KD, N], BF16, tag="xsb")
            # v resident in SBUF, [m-part, m-block, head, 65]; the 65th
            # column stays 1.0 so PV also accumulates the softmax denom
            v_sb = cpool.tile([128, MB, HG, 65], BF16, tag="vsb")
            nc.vector.memset(v_sb[:], 1.0)

            # ---- per-pair q/k projection blocks (emitted interleaved with
            #      the previous pair's attention so the PE never idles) ----
            def proj_pair_blocks(p):
                csl = slice(p * 128, (p + 1) * 128)
                wt = {}

                def load_w():
                    for nm, wd in (("q", wq), ("k", wk)):
                        t = iopool.tile(
                            [128, KD, 128], BF16, tag=f"w{nm}", bufs=1,
                            name=f"w{nm}_{p}",
                        )
                        nc.gpsimd.dma_start(
                            t[:], wd.rearrange("(c p) i -> p c i", p=128)[:, :, csl]
                        )
                        wt[nm] = t

                qT_t = qkpool.tile([128, N], BF16, tag="qT", bufs=2)
                kT_t = qkpool.tile([128, N], BF16, tag="kT", bufs=2)

                def block(nb, nm, tgt):
                    def emit():
                        nsl = slice(nb * 512, (nb + 1) * 512)
                        pq = ps.tile([128, 2, 512], F32, tag="s", bufs=3, name="pq")
                        for dc in range(KD):
                            nc.tensor.matmul(
                                pq[:, 0, :], wt[nm][:, dc, :], x_sb[:, dc, nsl],
                                start=(dc == 0), stop=(dc == KD - 1),
                            )
                        # rotate_half via 32-partition shifted copies; sign
                        # folded into sin_t (host negates low half rows)
                        q0 = iopool.tile([128, 512], BF16, tag="q0", bufs=2, name="q0")
                        nc.vector.tensor_copy(q0[:], pq[:, 0, :])
                        rot = iopool.tile([128, 512], BF16, tag="rot", bufs=2, name="rot")
                        for g in range(4):
                            dst = slice(g * 32, (g + 1) * 32)
                            ssrc = slice((g ^ 1) * 32, ((g ^ 1) + 1) * 32)
                            nc.vector.tensor_copy(rot[dst, :], q0[ssrc, :])
                        tmp = iopool.tile([128, 512], BF16, tag="tmp", bufs=2, name="tmp")
                        nc.vector.tensor_mul(tmp[:], rot[:], sin_t[:, nsl])
                        nc.vector.tensor_mul(tgt[:, nsl], q0[:], cos_t[:, nsl])
                        nc.vector.tensor_add(tgt[:, nsl], tgt[:, nsl], tmp[:])

                    return emit

                blocks = []
                for nb in range(NB):
                    blocks.append(block(nb, "q", qT_t))
                    blocks.append(block(nb, "k", kT_t))
                return load_w, blocks, qT_t, kT_t

            load_w0, blocks0, qT0, kT0 = proj_pair_blocks(0)
            load_w0()

            # ---- first pass over x: v projection (all heads) + pair-0 q/k ----
            with tc.tile_pool(name="vproj", bufs=1) as vpj:
                wv_t = vpj.tile([128, KD, INNER], BF16, tag="wv")
                wvr = wv.rearrange("(c p) i -> p c i", p=128)
                for nb in range(NB):
                    nsl = slice(nb * 512, (nb + 1) * 512)
                    if nb == 0:
                        for dc in range(KD):
                            nc.gpsimd.dma_start(wv_t[:, dc, :], wvr[:, dc, :])
                    nc.sync.dma_start(x_sb[:, :, nsl], xTr[:, :, nsl])
                    for sub in range(4):
                        pv = ps.tile([128, 512], F32, tag="s", bufs=3, name="pv")
                        m0 = nb * 512 + sub * 128
                        for dc in range(KD):
                            nc.tensor.matmul(
                                pv[:],
                                x_sb[:, dc, m0 : m0 + 128],
                                wv_t[:, dc, :],
                                start=(dc == 0),
                                stop=(dc == KD - 1),
                            )
                        nc.vector.tensor_copy(
                            v_sb[:, nb * 4 + sub, :, 0:64],
                            pv.rearrange("p (h d) -> p h d", h=HG),
                        )
                    blocks0[2 * nb]()
                    blocks0[2 * nb + 1]()

            pair_qk = {0: (qT0, kT0)}

            # ---- attention (pair p) interleaved with projections (p+1) ----
            with tc.tile_pool(name="attn", bufs=1) as at:
                otn = [
                    at.tile([128, 4, 512], BF16, tag=f"otn{p}", name=f"otn{p}")
                    for p in range(PAIRS)
                ]
                wo_h = []

                def load_wo():
                    for dh, wtag in ((0, "qT"), (1, "kT")):
                        woh = qkpool.tile(
                            [128, PAIRS, 512], BF16, tag=wtag, bufs=2,
                            name=f"wo_h{dh}",
                        )
                        nc.gpsimd.dma_start(
                            woh[:],
                            wo.rearrange("(c p) d -> p c d", p=128)[
                                :, :, dh * 512 : (dh + 1) * 512
                            ],
                        )
                        wo_h.append(woh)

                opq = []
                nmq = []

                def outproj_block(nb, dh):
                    def emit():
                        q4, r4 = divmod(nb, 4)
                        nsl = slice(nb * 128, (nb + 1) * 128)
                        po = ps.tile([128, 2, 512], F32, tag="s", bufs=3, name="po")
                        for c in range(PAIRS):
                            nc.tensor.matmul(
                                po[:, 0, :],
                                otn[c][:, q4, r4 * 128 : (r4 + 1) * 128],
                                wo_h[dh][:, c, :],
                                start=(c == 0),
                                stop=(c == PAIRS - 1),
                            )
                        ost = iopool.tile([128, 512], BF16, tag="ost", bufs=2, name="ost")
                        nc.any.tensor_copy(ost[:], po[:, 0, :])
                        nc.sync.dma_start(
                            out[nsl, dh * 512 : (dh + 1) * 512], ost[:]
                        )

                    return emit

                def outproj_quarter(q4):
                    # queue this quarter's out-projection; drained a few
                    # blocks at a time inside the next quarter's loop
                    for r4 in range(4):
                        for dh in range(2):
                            opq.append(outproj_block(q4 * 4 + r4, dh))

                # `pending` carries each quarter's last PV pair + tail (ot
                # spill, denominators, reciprocal) into the NEXT quarter's
                # first iteration, so the PE stream never waits on them
                pending = []

                for p in range(PAIRS):
                    qT_t, kT_t = pair_qk.pop(p)
                    if p == PAIRS - 1:
                        load_wo()
                    if p + 1 < PAIRS:
                        load_wn, blocks_n, qTn, kTn = proj_pair_blocks(p + 1)
                        load_wn()
                        pair_qk[p + 1] = (qTn, kTn)
                    else:
                        blocks_n = []
                    blk_i = 0
                    for f in range(2):
                        for sub in range(2):
                            n0 = f * 1024 + sub * 512
                            ot_ab = [
                                ps.tile([128, 512], F32, tag="ot", bufs=2, name=f"ot{jj}")
                                for jj in range(2)
                            ]
                            pv_prev = None
                            for mb2 in range(MB // 2):
                                s_tiles = []
                                for j in range(2):
                                    psl = slice(64 * j, 64 * (j + 1))
                                    s_t = ps.tile([128, 2, 512], F32, tag="s", bufs=3, name=f"s{j}")
                                    for hm in range(2):
                                        mb = 2 * mb2 + hm
                                        msl = slice(mb * 128, (mb + 1) * 128)
                                        nc.tensor.matmul(
                                            s_t[:, hm, :],
                                            kT_t[psl, msl],
                                            qT_t[psl, n0 : n0 + 512],
                                            start=True,
                                            stop=True,
                                        )
                                    s_tiles.append(s_t)
                                pts = []
                                for j in range(2):
                                    pt = at.tile([128, 2, 512], BF16, tag="pt", bufs=5, name=f"pt{j}")
                                    nc.scalar.activation(
                                        pt[:], s_tiles[j][:], EXP, scale=SCALE
                                    )
                                    pts.append(pt)
                                # previous quarter's last PV + tail first ...
                                if mb2 == 0 and pending:
                                    pending.pop(0)()
                                # ... then this quarter's PV, one iteration
                                # behind its exp so the PE never waits on it
                                if pv_prev is not None:
                                    pv_prev()

                                def make_pv(mb2=mb2, pts=pts, ot_ab=ot_ab, p=p):
                                    def emit_pv():
                                        for j in range(2):
                                            for hm in range(2):
                                                mb = 2 * mb2 + hm
                                                nc.tensor.matmul(
                                                    ot_ab[j][0:65, :],
                                                    v_sb[:, mb, 2 * p + j, :],
                                                    pts[j][:, hm, :],
                                                    start=(mb == 0),
                                                    stop=(mb == MB - 1),
                                                )

                                    return emit_pv

                                pv_prev = make_pv()
                                # previous quarter's deferred normalize: its
                                # reciprocal (emitted at mb2==0) has had ~3
                                # iterations to finish. Both head halves must
                                # drain before any outproj pop below reads
                                # otn (write-after-read hazard).
                                if mb2 in (3, 4) and nmq:
                                    nmq.pop(0)()
                                # spread next pair's projection work through
                                # the attention chain to keep the PE dense
                                if mb2 % 2 == 1:
                                    if blk_i < len(blocks_n):
                                        blocks_n[blk_i]()
                                    blk_i += 1
                                    # in the last pair, spread the previous
                                    # quarter's output projection here too
                                    if mb2 >= 5:
                                        for _ in range(4):
                                            if opq:
                                                opq.pop(0)()

                            def make_tail(
                                pv_last=pv_prev, ot_ab=ot_ab, p=p, f=f, sub=sub
                            ):
                                def emit_tail():
                                    pv_last()
                                    # spill OT accumulators to SBUF (frees
                                    # the psum banks for the next quarter)
                                    osb = at.tile([65, 2, 512], BF16, tag="ots", bufs=4)
                                    nc.vector.tensor_copy(osb[:, 0, :], ot_ab[0][0:65, :])
                                    nc.vector.tensor_copy(osb[:, 1, :], ot_ab[1][0:65, :])
                                    rin = at.tile([33, 512], F32, tag="rin", bufs=2)
                                    nc.vector.tensor_copy(rin[0:1, :], osb[64:65, 0, :])
                                    nc.vector.tensor_copy(rin[32:33, :], osb[64:65, 1, :])
                                    rec = at.tile([33, 512], F32R, tag="rec", bufs=2)
                                    with nc.allow_low_precision(
                                        reason="f32r reciprocal for softmax denom"
                                    ):
                                        # one op covers rows 0..32; 1-31 junk
                                        nc.vector.reciprocal(rec[:], rin[:])

                                    def norm_emit(j, osb=osb, rec=rec):
                                        row = 32 * j
                                        bc = ps.tile(
                                            [128, 2, 512], F32, tag="s", bufs=3,
                                            name=f"bc{j}",
                                        )
                                        nc.tensor.matmul(
                                            bc[0:64, 0, :],
                                            onesr[row : row + 1, :],
                                            rec[row : row + 1, :],
                                            start=True,
                                            stop=True,
                                        )
                                        nc.vector.tensor_mul(
                                            otn[p][64 * j : 64 * (j + 1), f * 2 + sub, :],
                                            osb[0:64, j, :],
                                            bc[0:64, 0, :],
                                        )

                                    nmq.append(lambda: norm_emit(0))
                                    nmq.append(lambda: norm_emit(1))
                                    if p == PAIRS - 1:
                                        outproj_quarter(f * 2 + sub)

                                return emit_tail

                            pending.append(make_tail())
                while pending:
                    pending.pop(0)()
                while nmq:
                    nmq.pop(0)()
                while opq:
                    opq.pop(0)()

    return nc


_CACHED = {}


def _get_kernel():
    if "nc" not in _CACHED:
        _CACHED["nc"] = build_kernel()
    return _CACHED["nc"]


def kernel(x, rotary_emb_x, Wq, Wkv, Wo, bo):
    import ml_dtypes

    from concourse.bass_utils import run_bass_kernel_spmd

    BF = ml_dtypes.bfloat16

    x = np.asarray(x, np.float32)
    rope = np.asarray(rotary_emb_x, np.float32)
    Wq = np.asarray(Wq, np.float32)
    Wkv = np.asarray(Wkv, np.float32)
    Wo = np.asarray(Wo, np.float32)
    bo = np.asarray(bo, np.float32)

    cosT = np.ascontiguousarray(np.cos(rope).T)  # [64, N]
    sinT = np.ascontiguousarray(np.sin(rope).T)
    cosT2 = np.concatenate([cosT, cosT], axis=0)
    sinT2 = np.concatenate([sinT, sinT], axis=0)
    # fold rotate_half's sign into sin: the low half of each 64-row head
    # block multiplies -q_hi
    sinT2 = sinT2.copy()
    sinT2[0:32] = -sinT2[0:32]
    sinT2[64:96] = -sinT2[64:96]
    cosT2 = cosT2.astype(BF)
    sinT2 = sinT2.astype(BF)

    Wk_full = Wkv[:, : H * DH]
    Wv_full = Wkv[:, H * DH :]

    xTs = [np.ascontiguousarray(x[b].T).astype(BF) for b in range(B)]
    in_maps = []
    for core in range(N_CORES):
        b, hg = divmod(core, 2)
        isl = slice(hg * INNER, (hg + 1) * INNER)
        in_maps.append(
            {
                "xT": xTs[b],
                "wq": np.ascontiguousarray(Wq[:, isl]).astype(BF),
                "wk": np.ascontiguousarray(Wk_full[:, isl]).astype(BF),
                "wv": np.ascontiguousarray(Wv_full[:, isl]).astype(BF),
                "wo": np.ascontiguousarray(Wo[isl, :]).astype(BF),
                "cosT": cosT2,
                "sinT": sinT2,
            }
        )

    nc = _get_kernel()
    _CACHED["in_maps"] = in_maps
    res = run_bass_kernel_spmd(nc, in_maps, list(range(N_CORES)))
    outs = [
        np.asarray(res.results[i]["out"]).astype(np.float32)
        for i in range(N_CORES)
    ]
    full = np.stack(
        [outs[2 * b] + outs[2 * b + 1] + bo for b in range(B)], axis=0
    )
    return full


# revision 12
# speedup vs baseline: 1.5964x; 1.0041x over previous
"""Multi-head self-attention (RoPE + softmax + out-proj) for Trainium2,
sharded over 8 NeuronCores: data-parallel over batch (4) x tensor-parallel
over heads (2 groups of 8). Each core computes q/k/v projections for its
head group, attention, and a partial output projection; the host sums the
two partials per batch and adds the bias.

Per-core layout highlights:
  - All matmul operands are bf16 (host pre-casts inputs), which streams at
    the full 1 cycle/row PE rate and halves SBUF/DMA traffic; PSUM
    accumulation stays fp32.
  - x^T stays resident in SBUF (no DRAM bounce); v is projected straight
    into a resident SBUF tile [128, mb, head, 65] whose 65th column is
    pre-set to 1 so each PV matmul also accumulates the softmax denominator
    in PSUM row 64.
  - q/k are produced transposed ([head_dim, n]); RoPE's rotate_half is done
    with 32-partition shifted bf16 copies, sign folded into a host-negated
    sin table.
  - Scores are computed transposed (S^T[m, n]) with K=64 row-group-packed
    matmul pairs (two heads concurrently in the PE array); exp runs on the
    scalar engine straight out of PSUM in 1024-wide instructions.
  - The softmax normalize (reciprocal -> PE broadcast -> multiply) is
    deferred one quarter so the PE never waits on the DVE chain; the
    attention wave of pair p is software-pipelined with the projections of
    pair p+1 and the output projection of finished quarters.
"""

import numpy as np

import concourse.bass as bass
import concourse.mybir as mybir
import concourse.tile as tile

B, N, DIM, H, DH = 4, 2048, 1024, 16, 64
SCALE = DH**-0.5
N_CORES = 8
HG = 8  # heads per core
INNER = HG * DH  # 512, inner dim slice per core
PAIRS = INNER // 128  # 4 head pairs (=128-partition inner chunks)
NB = 4  # n blocks of 512
MB = 16  # m blocks of 128
KD = DIM // 128  # 8 contraction chunks

F32 = mybir.dt.float32
F32R = mybir.dt.float32r
BF16 = mybir.dt.bfloat16
EXP = mybir.ActivationFunctionType.Exp

MAX_WAITS = 1


def _split_excess_waits(nc):
    """This walrus build rejects >1 semaphore wait per instruction; hoist
    excess waits onto nops inserted before the instruction on its engine."""
    import bass_rust

    for f in nc.m.functions:
        for bb in f.blocks:
            il = bb.instructions
            i = 0
            while i < len(il):
                inst = il[i]
                si = inst.sync_info
                if si is not None and si.on_wait and len(si.on_wait) > MAX_WAITS:
                    waits = list(si.on_wait)
                    si.on_wait = waits[:MAX_WAITS]
                    rest = waits[MAX_WAITS:]
                    eng = nc.engines[inst.engine]
                    insert_at = i
                    for j in range(0, len(rest), MAX_WAITS):
                        b = eng.nop(nofuse=True, hint="wait_split")
                        ni = b.ins
                        tail = nc.cur_bb.bb.instructions
                        assert tail[-1] is ni
                        tail.pop()
                        nsi = ni.sync_info
                        if nsi is None:
                            ni.sync_info = bass_rust.SyncInfo(
                                on_wait=rest[j : j + MAX_WAITS], on_update=[]
                            )
                        else:
                            nsi.on_wait = rest[j : j + MAX_WAITS]
                        il.insert(insert_at, ni)
                        insert_at += 1
                        i += 1
                i += 1


class _FixedTileContext(tile.TileContext):
    def __exit__(self, exc_type, exc_val, exc_tb):
        res = super().__exit__(exc_type, exc_val, exc_tb)
        if exc_type is None:
            _split_excess_waits(self.nc)
        return res


def build_kernel():
    nc = bass.Bass()
    xT = nc.dram_tensor("xT", [DIM, N], BF16, kind="ExternalInput")
    wq = nc.dram_tensor("wq", [DIM, INNER], BF16, kind="ExternalInput")
    wk = nc.dram_tensor("wk", [DIM, INNER], BF16, kind="ExternalInput")
    wv = nc.dram_tensor("wv", [DIM, INNER], BF16, kind="ExternalInput")
    wo = nc.dram_tensor("wo", [INNER, DIM], BF16, kind="ExternalInput")
    cosT = nc.dram_tensor("cosT", [128, N], BF16, kind="ExternalInput")
    sinT = nc.dram_tensor("sinT", [128, N], BF16, kind="ExternalInput")
    out = nc.dram_tensor("out", [N, DIM], BF16, kind="ExternalOutput")

    xTr = xT.rearrange("(c p) n -> p c n", p=128)

    with _FixedTileContext(nc) as tc:
        with (
            tc.tile_pool(name="const", bufs=1) as cpool,
            tc.tile_pool(name="qk", bufs=1) as qkpool,
            tc.tile_pool(name="ps", space=bass.MemorySpace.PSUM, bufs=1) as ps,
            tc.tile_pool(name="io", bufs=1) as iopool,
        ):
            # ---- constants / resident tensors ----
            cos_t = cpool.tile([128, N], BF16, tag="cos")
            sin_t = cpool.tile([128, N], BF16, tag="sin")
            nc.sync.dma_start(cos_t[:], cosT[:])
            nc.sync.dma_start(sin_t[:], sinT[:])
            ones_f = cpool.tile([128, 64], F32, tag="onesf")
            nc.vector.memset(ones_f[:], 1.0)
            onesr = cpool.tile([128, 64], F32R, tag="onesr")
            nc.vector.tensor_copy(onesr[:], ones_f[:])
            # x^T resident in SBUF (bf16): DMA'd in 512-col chunks below
            x_sb = cpool.tile([128, KD, N], BF16, tag="xsb")
            # v resident in SBUF, [m-part, m-block, head, 65]; the 65th
            # column stays 1.0 so PV also accumulates the softmax denom
            v_sb = cpool.tile([128, MB, HG, 65], BF16, tag="vsb")
            nc.vector.memset(v_sb[:], 1.0)

            # ---- per-pair q/k projection blocks (emitted interleaved with
            #      the previous pair's attention so the PE never idles) ----
            def proj_pair_blocks(p):
                csl = slice(p * 128, (p + 1) * 128)
                wt = {}

                def load_w():
                    for nm, wd in (("q", wq), ("k", wk)):
                        t = iopool.tile(
                            [128, KD, 128], BF16, tag=f"w{nm}", bufs=1,
                            name=f"w{nm}_{p}",
                        )
                        nc.gpsimd.dma_start(
                            t[:], wd.rearrange("(c p) i -> p c i", p=128)[:, :, csl]
                        )
                        wt[nm] = t

                qT_t = qkpool.tile([128, N], BF16, tag="qT", bufs=2)
                kT_t = qkpool.tile([128, N], BF16, tag="kT", bufs=2)

                def block(nb, nm, tgt):
                    def emit():
                        nsl = slice(nb * 512, (nb + 1) * 512)
                        pq = ps.tile([128, 2, 512], F32, tag="s", bufs=3, name="pq")
                        for dc in range(KD):
                            nc.tensor.matmul(
                                pq[:, 0, :], wt[nm][:, dc, :], x_sb[:, dc, nsl],
                                start=(dc == 0), stop=(dc == KD - 1),
                            )
                        # rotate_half via 32-partition shifted copies; sign
                        # folded into sin_t (host negates low half rows)
                        q0 = iopool.tile([128, 512], BF16, tag="q0", bufs=2, name="q0")
                        nc.vector.tensor_copy(q0[:], pq[:, 0, :])
                        rot = iopool.tile([128, 512], BF16, tag="rot", bufs=2, name="rot")
                        for g in range(4):
                            dst = slice(g * 32, (g + 1) * 32)
                            ssrc = slice((g ^ 1) * 32, ((g ^ 1) + 1) * 32)
                            nc.vector.tensor_copy(rot[dst, :], q0[ssrc, :])
                        tmp = iopool.tile([128, 512], BF16, tag="tmp", bufs=2, name="tmp")
                        nc.vector.tensor_mul(tmp[:], rot[:], sin_t[:, nsl])
                        nc.vector.tensor_mul(tgt[:, nsl], q0[:], cos_t[:, nsl])
                        nc.vector.tensor_add(tgt[:, nsl], tgt[:, nsl], tmp[:])

                    return emit

                blocks = []
                for nb in range(NB):
                    blocks.append(block(nb, "q", qT_t))
                    blocks.append(block(nb, "k", kT_t))
                return load_w, blocks, qT_t, kT_t

            load_w0, blocks0, qT0, kT0 = proj_pair_blocks(0)
            load_w0()

            # ---- first pass over x: v projection (all heads) + pair-0 q/k,
            #      with quarter-0 scores+exp streamed in as k blocks land so
            #      the scalar engine starts working ~10us in ----
            q0pts = []

            def q0_scores(mb2):
                s_tiles = []
                for j in range(2):
                    psl = slice(64 * j, 64 * (j + 1))
                    s_t = ps.tile([128, 2, 512], F32, tag="s", bufs=3, name=f"s{j}")
                    for hm in range(2):
                        mb = 2 * mb2 + hm
                        msl = slice(mb * 128, (mb + 1) * 128)
                        nc.tensor.matmul(
                            s_t[:, hm, :],
                            kT0[psl, msl],
                            qT0[psl, 0:512],
                            start=True,
                            stop=True,
                        )
                    s_tiles.append(s_t)
                pts = []
                for j in range(2):
                    pt = cpool.tile(
                        [128, 2, 512], BF16, tag=f"q0pt{mb2}_{j}",
                        name=f"q0pt{mb2}_{j}",
                    )
                    nc.scalar.activation(pt[:], s_tiles[j][:], EXP, scale=SCALE)
                    pts.append(pt)
                q0pts.append(pts)

            with tc.tile_pool(name="vproj", bufs=1) as vpj:
                wv_t = vpj.tile([128, KD, INNER], BF16, tag="wv")
                wvr = wv.rearrange("(c p) i -> p c i", p=128)
                for nb in range(NB):
                    nsl = slice(nb * 512, (nb + 1) * 512)
                    if nb == 0:
                        for dc in range(KD):
                            nc.gpsimd.dma_start(wv_t[:, dc, :], wvr[:, dc, :])
                    nc.sync.dma_start(x_sb[:, :, nsl], xTr[:, :, nsl])
                    blocks0[2 * nb + 1]()  # k(nb) first: gates scores
                    blocks0[2 * nb]()  # q(nb); scores only need q(0)
                    for sub in range(4):
                        pv = ps.tile([128, 512], F32, tag="s", bufs=3, name="pv")
                        m0 = nb * 512 + sub * 128
                        for dc in range(KD):
                            nc.tensor.matmul(
                                pv[:],
                                x_sb[:, dc, m0 : m0 + 128],
                                wv_t[:, dc, :],
                                start=(dc == 0),
                                stop=(dc == KD - 1),
                            )
                        nc.vector.tensor_copy(
                            v_sb[:, nb * 4 + sub, :, 0:64],
                            pv.rearrange("p (h d) -> p h d", h=HG),
                        )
                        if sub == 1:
                            q0_scores(2 * nb)
                        elif sub == 3:
                            q0_scores(2 * nb + 1)

            pair_qk = {0: (qT0, kT0)}

            # ---- attention (pair p) interleaved with projections (p+1) ----
            with tc.tile_pool(name="attn", bufs=1) as at:
                otn = [
                    at.tile([128, 4, 512], BF16, tag=f"otn{p}", name=f"otn{p}")
                    for p in range(PAIRS)
                ]
                wo_h = []

                def load_wo():
                    for dh, wtag in ((0, "qT"), (1, "kT")):
                        woh = qkpool.tile(
                            [128, PAIRS, 512], BF16, tag=wtag, bufs=2,
                            name=f"wo_h{dh}",
                        )
                        nc.gpsimd.dma_start(
                            woh[:],
                            wo.rearrange("(c p) d -> p c d", p=128)[
                                :, :, dh * 512 : (dh + 1) * 512
                            ],
                        )
                        wo_h.append(woh)

                opq = []
                nmq = []

                def outproj_block(nb, dh):
                    def emit():
                        q4, r4 = divmod(nb, 4)
                        nsl = slice(nb * 128, (nb + 1) * 128)
                        po = ps.tile([128, 2, 512], F32, tag="s", bufs=3, name="po")
                        for c in range(PAIRS):
                            nc.tensor.matmul(
                                po[:, 0, :],
                                otn[c][:, q4, r4 * 128 : (r4 + 1) * 128],
                                wo_h[dh][:, c, :],
                                start=(c == 0),
                                stop=(c == PAIRS - 1),
                            )
                        ost = iopool.tile([128, 512], BF16, tag="ost", bufs=2, name="ost")
                        nc.any.tensor_copy(ost[:], po[:, 0, :])
                        nc.sync.dma_start(
                            out[nsl, dh * 512 : (dh + 1) * 512], ost[:]
                        )

                    return emit

                def outproj_quarter(q4):
                    # queue this quarter's out-projection; drained a few
                    # blocks at a time inside the next quarter's loop
                    for r4 in range(4):
                        for dh in range(2):
                            opq.append(outproj_block(q4 * 4 + r4, dh))

                # `pending` carries each quarter's last PV pair + tail (ot
                # spill, denominators, reciprocal) into the NEXT quarter's
                # first iteration, so the PE stream never waits on them
                pending = []

                for p in range(PAIRS):
                    qT_t, kT_t = pair_qk.pop(p)
                    if p == PAIRS - 1:
                        load_wo()
                    if p + 1 < PAIRS:
                        load_wn, blocks_n, qTn, kTn = proj_pair_blocks(p + 1)
                        load_wn()
                        pair_qk[p + 1] = (qTn, kTn)
                    else:
                        blocks_n = []
                    blk_i = 0
                    for f in range(2):
                        for sub in range(2):
                            n0 = f * 1024 + sub * 512
                            ot_ab = [
                                ps.tile([128, 512], F32, tag="ot", bufs=2, name=f"ot{jj}")
                                for jj in range(2)
                            ]
                            pv_prev = None
                            for mb2 in range(MB // 2):
                                if p == 0 and f == 0 and sub == 0:
                                    # scores+exp were streamed during the
                                    # first pass; only PV remains here
                                    pts = q0pts[mb2]
                                else:
                                    s_tiles = []
                                    for j in range(2):
                                        psl = slice(64 * j, 64 * (j + 1))
                                        s_t = ps.tile([128, 2, 512], F32, tag="s", bufs=3, name=f"s{j}")
                                        for hm in range(2):
                                            mb = 2 * mb2 + hm
                                            msl = slice(mb * 128, (mb + 1) * 128)
                                            nc.tensor.matmul(
                                                s_t[:, hm, :],
                                                kT_t[psl, msl],
                                                qT_t[psl, n0 : n0 + 512],
                                                start=True,
                                                stop=True,
                                            )
                                        s_tiles.append(s_t)
                                    pts = []
                                    for j in range(2):
                                        pt = at.tile([128, 2, 512], BF16, tag="pt", bufs=5, name=f"pt{j}")
                                        nc.scalar.activation(
                                            pt[:], s_tiles[j][:], EXP, scale=SCALE
                                        )
                                        pts.append(pt)
                                # previous quarter's last PV + tail first ...
                                if mb2 == 0 and pending:
                                    pending.pop(0)()
                                # ... then this quarter's PV, one iteration
                                # behind its exp so the PE never waits on it
                                if pv_prev is not None:
                                    pv_prev()

                                def make_pv(mb2=mb2, pts=pts, ot_ab=ot_ab, p=p):
                                    def emit_pv():
                                        for j in range(2):
                                            for hm in range(2):
                                                mb = 2 * mb2 + hm
                                                nc.tensor.matmul(
                                                    ot_ab[j][0:65, :],
                                                    v_sb[:, mb, 2 * p + j, :],
                                                    pts[j][:, hm, :],
                                                    start=(mb == 0),
                                                    stop=(mb == MB - 1),
                                                )

                                    return emit_pv

                                pv_prev = make_pv()
                                # previous quarter's deferred normalize: its
                                # reciprocal (emitted at mb2==0) has had ~3
                                # iterations to finish. Both head halves must
                                # drain before any outproj pop below reads
                                # otn (write-after-read hazard).
                                if mb2 in (3, 4) and nmq:
                                    nmq.pop(0)()
                                # spread next pair's projection work through
                                # the attention chain to keep the PE dense
                                if mb2 % 2 == 1:
                                    if blk_i < len(blocks_n):
                                        blocks_n[blk_i]()
                                    blk_i += 1
                                    # in the last pair, spread the previous
                                    # quarter's output projection here too
                                    if mb2 >= 5:
                                        for _ in range(4):
                                            if opq:
                                                opq.pop(0)()

                            def make_tail(
                                pv_last=pv_prev, ot_ab=ot_ab, p=p, f=f, sub=sub
                            ):
                                def emit_tail():
                                    pv_last()
                                    # spill OT accumulators to SBUF (frees
                                    # the psum banks for the next quarter)
                                    osb = at.tile([65, 2, 512], BF16, tag="ots", bufs=4)
                                    nc.vector.tensor_copy(osb[:, 0, :], ot_ab[0][0:65, :])
                                    nc.vector.tensor_copy(osb[:, 1, :], ot_ab[1][0:65, :])
                                    rin = at.tile([33, 512], F32, tag="rin", bufs=2)
                                    nc.vector.tensor_copy(rin[0:1, :], osb[64:65, 0, :])
                                    nc.vector.tensor_copy(rin[32:33, :], osb[64:65, 1, :])
                                    rec = at.tile([33, 512], F32R, tag="rec", bufs=2)
                                    with nc.allow_low_precision(
                                        reason="f32r reciprocal for softmax denom"
                                    ):
                                        # one op covers rows 0..32; 1-31 junk
                                        nc.vector.reciprocal(rec[:], rin[:])

                                    def norm_emit(j, osb=osb, rec=rec):
                                        row = 32 * j
                                        bc = ps.tile(
                                            [128, 2, 512], F32, tag="s", bufs=3,
                                            name=f"bc{j}",
                                        )
                                        nc.tensor.matmul(
                                            bc[0:64, 0, :],
                                            onesr[row : row + 1, :],
                                            rec[row : row + 1, :],
                                            start=True,
                                            stop=True,
                                        )
                                        nc.vector.tensor_mul(
                                            otn[p][64 * j : 64 * (j + 1), f * 2 + sub, :],
                                            osb[0:64, j, :],
                                            bc[0:64, 0, :],
                                        )

                                    nmq.append(lambda: norm_emit(0))
                                    nmq.append(lambda: norm_emit(1))
                                    if p == PAIRS - 1:
                                        outproj_quarter(f * 2 + sub)

                                return emit_tail

                            pending.append(make_tail())
                while pending:
                    pending.pop(0)()
                while nmq:
                    nmq.pop(0)()
                while opq:
                    opq.pop(0)()

    return nc


_CACHED = {}


def _get_kernel():
    if "nc" not in _CACHED:
        _CACHED["nc"] = build_kernel()
    return _CACHED["nc"]


def kernel(x, rotary_emb_x, Wq, Wkv, Wo, bo):
    import ml_dtypes

    from concourse.bass_utils import run_bass_kernel_spmd

    BF = ml_dtypes.bfloat16

    x = np.asarray(x, np.float32)
    rope = np.asarray(rotary_emb_x, np.float32)
    Wq = np.asarray(Wq, np.float32)
    Wkv = np.asarray(Wkv, np.float32)
    Wo = np.asarray(Wo, np.float32)
    bo = np.asarray(bo, np.float32)

    cosT = np.ascontiguousarray(np.cos(rope).T)  # [64, N]
    sinT = np.ascontiguousarray(np.sin(rope).T)
    cosT2 = np.concatenate([cosT, cosT], axis=0)
    sinT2 = np.concatenate([sinT, sinT], axis=0)
    # fold rotate_half's sign into sin: the low half of each 64-row head
    # block multiplies -q_hi
    sinT2 = sinT2.copy()
    sinT2[0:32] = -sinT2[0:32]
    sinT2[64:96] = -sinT2[64:96]
    cosT2 = cosT2.astype(BF)
    sinT2 = sinT2.astype(BF)

    Wk_full = Wkv[:, : H * DH]
    Wv_full = Wkv[:, H * DH :]

    xTs = [np.ascontiguousarray(x[b].T).astype(BF) for b in range(B)]
    in_maps = []
    for core in range(N_CORES):
        b, hg = divmod(core, 2)
        isl = slice(hg * INNER, (hg + 1) * INNER)
        in_maps.append(
            {
                "xT": xTs[b],
                "wq": np.ascontiguousarray(Wq[:, isl]).astype(BF),
                "wk": np.ascontiguousarray(Wk_full[:, isl]).astype(BF),
                "wv": np.ascontiguousarray(Wv_full[:, isl]).astype(BF),
                "wo": np.ascontiguousarray(Wo[isl, :]).astype(BF),
                "cosT": cosT2,
                "sinT": sinT2,
            }
        )

    nc = _get_kernel()
    _CACHED["in_maps"] = in_maps
    res = run_bass_kernel_spmd(nc, in_maps, list(range(N_CORES)))
    outs = [
        np.asarray(res.results[i]["out"]).astype(np.float32)
        for i in range(N_CORES)
    ]
    full = np.stack(
        [outs[2 * b] + outs[2 * b + 1] + bo for b in range(B)], axis=0
    )
    return full
